# revision 24
# baseline (speedup 1.0000x reference)
"""Trainium2 Bass kernel for causal multi-head attention (B=4, T=2048, C=1024, H=16).

Sharding: 8 NeuronCores = batch (4) x head-group (2). Each core computes, for
its batch b and its 8 heads:
  - QKV projections with column-sharded weights (Q^T/K^T in [D*,T] layout,
    V in [T, D*] layout),
  - causal attention with an appended validity/row-sum column on V
    (flash-style unnormalized accumulation + fused denominator),
  - row-sharded output projection producing a partial [T, C] output.
The host sums the two head-group partials per batch and adds the output bias.

All matmul operands are bf16 (fp32 PSUM accumulation; rel err ~5e-3, well
inside the 2e-2 gate); operands are cast host-side. Weight/x-block loads are
single batched DMAs (3D access patterns), y^T stays resident in SBUF (no
DRAM bounce), and the per-row-tile output is written with one DMA — DMA
issue on the sync queue was the serial bottleneck at 204 DMAs/exec.

Schedule: one rolling loop — attention for query-block qb is emitted
interleaved with the projections of block qb+1, so the exp-bound Scalar-engine
stretches hide under PE-bound projection matmuls (Q^T rolls per block; the
output projection reads SBUF-resident y^T and interleaves with the last
block's attention). Attention-V matmuls lag the score/exp stream (AV_LAG) to
keep the in-order PE queue from head-of-line blocking on the Scalar engine.

`_build(t_len, repeat=K)` emits the whole body K times into one NEFF; test.py
uses that for loop-timing on hardware (the axon tunnel's ~2-15 ms per-call
dispatch floor would otherwise swamp the ~0.44 ms device time).
"""

import numpy as np
from contextlib import ExitStack

B, T, C, H = 4, 2048, 1024, 16
D = C // H            # 64
CL = C // 2           # 512 local channels (8 heads) per core
NCI = C // 128        # 8 contraction tiles for projections
PAIR_BLK = 192        # v_sb columns per head pair: [V_e(64) | valid(1) | gap(63) | V_o(64)]

_CACHE = {}

# schedule-pipelining knobs
AV_LAG = 10
ST_BUFS = 2
ES_BUFS = 12

# timing-variant switches (bench-only; kernel() always runs the full body)
_BENCH = dict(scores=True, mask=True, exp=True, av=True, norm=True,
              spill=True, proj=True)

# matmul-operand precision: "bf16" (half SBUF/DMA traffic) or "f32r"
_PREC = "bf16"


def _f32r_round(a):
    """Round fp32 -> float32r bit pattern (keep top 12 mantissa bits, round half up)."""
    a = np.ascontiguousarray(a, dtype=np.float32)
    u = a.view(np.uint32).astype(np.uint64)
    r = ((u + 0x7FF + ((u >> 12) & 1)) & 0xFFFFF000).astype(np.uint32)
    return r.view(np.float32).reshape(a.shape)


def _cast_op(a):
    """Host-side cast of a matmul operand to the active precision."""
    if _PREC == "bf16":
        import ml_dtypes
        return np.ascontiguousarray(a, dtype=np.float32).astype(ml_dtypes.bfloat16)
    return _f32r_round(a)


def _build(t_len, repeat=1):
    import concourse.bass as bass  # noqa: F401
    import concourse.tile as tile
    from concourse import bacc, mybir

    dt = mybir.dt
    AF = mybir.ActivationFunctionType
    Alu = mybir.AluOpType

    NT = t_len // 128     # t tiles
    NB = t_len // 512     # t blocks

    dta = dt.bfloat16 if _PREC == "bf16" else dt.float32r

    nc = bacc.Bacc("TRN2", target_bir_lowering=False, debug=False,
                   enable_asserts=False, num_devices=8)

    xt_d = nc.dram_tensor("xt", (C, t_len), dta, kind="ExternalInput").ap()
    wq_d = nc.dram_tensor("wq", (C, CL), dta, kind="ExternalInput").ap()
    wk_d = nc.dram_tensor("wk", (C, CL), dta, kind="ExternalInput").ap()
    wv_d = nc.dram_tensor("wv", (C, CL), dta, kind="ExternalInput").ap()
    wp_d = nc.dram_tensor("wp", (CL, C), dta, kind="ExternalInput").ap()
    bq_d = nc.dram_tensor("bq", (CL, 1), dt.float32, kind="ExternalInput").ap()
    bk_d = nc.dram_tensor("bk", (CL, 1), dt.float32, kind="ExternalInput").ap()
    bvr_d = nc.dram_tensor("bvr", (1, CL), dta, kind="ExternalInput").ap()
    vm_d = nc.dram_tensor("vm", (128, NT), dt.float32, kind="ExternalInput").ap()
    mka_d = nc.dram_tensor("mka", (128, 256), dt.float32, kind="ExternalInput").ap()
    mkb_d = nc.dram_tensor("mkb", (128, 256), dta, kind="ExternalInput").ap()
    ones_d = nc.dram_tensor("ones", (128, 128), dta, kind="ExternalInput").ap()
    out_d = nc.dram_tensor("out", (t_len, C), dt.float32, kind="ExternalOutput").ap()

    with tile.TileContext(nc) as tc:
        for _rep in range(repeat):
            _emit_body(nc, tile, dt, AF, Alu, NT, NB, t_len, _rep,
                       xt_d, wq_d, wk_d, wv_d, wp_d, bq_d, bk_d, bvr_d,
                       vm_d, mka_d, ones_d, out_d, tc)

    nc.compile()
    return nc


def _emit_body(nc, tile, dt, dta, AF, Alu, NT, NB, t_len, rep,
               xt_d, wq_d, wk_d, wv_d, wp_d, bq_d, bk_d, bvr_d,
               vm_d, mka_d, mkb_d, ones_d, out_d, tc):
    with ExitStack() as octx:
        persist = octx.enter_context(tc.tile_pool(name=f"persist{rep}", bufs=1))

        # Small persistent tensors
        maskadd = persist.tile([128, 256], dt.float32, tag="mka")
        nc.sync.dma_start(maskadd[:], mka_d[:])
        mask01 = persist.tile([128, 256], dta, tag="mkb")
        nc.sync.dma_start(mask01[:], mkb_d[:])
        ones = persist.tile([128, 128], dta, tag="ones")
        nc.sync.dma_start(ones[:], ones_d[:])
        vm16 = persist.tile([128, NT], dt.float32, tag="vm16")
        nc.sync.dma_start(vm16[:], vm_d[:])
        bvr = persist.tile([1, CL], dta, tag="bvr")
        nc.sync.dma_start(bvr[:], bvr_d[:])
        bq_sb = persist.tile([128, 4], dt.float32, tag="bq")
        bk_sb = persist.tile([128, 4], dt.float32, tag="bk")
        nc.sync.dma_start(bq_sb[:].rearrange("p (j one) -> p j one", one=1),
                          bq_d[:].rearrange("(j p) one -> p j one", p=128))
        nc.sync.dma_start(bk_sb[:].rearrange("p (j one) -> p j one", one=1),
                          bk_d[:].rearrange("(j p) one -> p j one", p=128))

        # Persistent activations (Q^T is rolled per t-block; K^T/V/y^T persist)
        kt_ = [persist.tile([128, t_len], dta, tag=f"kt{j}", name=f"kt{j}") for j in range(4)]
        vsb = [persist.tile([128, 4 * PAIR_BLK], dta, tag=f"v{t}",
                             name=f"v{t}") for t in range(NT)]
        ysb = [persist.tile([128, t_len], dta, tag=f"y{j}", name=f"ysb{j}")
               for j in range(4)]

        # ------- merged loop: projections for t-block tb, then attention qb=tb -------
        with (
            tc.tile_pool(name="pm", bufs=1) as pm,
            tc.tile_pool(name="psm", bufs=1, space="PSUM") as psm,
        ):
            # one batched DMA per weight matrix / x block
            wq_all = pm.tile([128, NCI * CL], dta, tag="wqa", name="wq_all")
            wk_all = pm.tile([128, NCI * CL], dta, tag="wka", name="wk_all")
            wv_all = pm.tile([128, NCI * CL], dta, tag="wva", name="wv_all")
            wp_all = pm.tile([128, 4 * C], dta, tag="wpa", name="wp_all")
            xs0 = pm.tile([128, NCI * 512], dta, tag="xall", name="xs0", bufs=2)
            nc.sync.dma_start(
                xs0[:].rearrange("p (ci c) -> p ci c", ci=NCI),
                xt_d[:, 0:512].rearrange("(ci p) c -> p ci c", p=128))
            nc.sync.dma_start(
                wq_all[:].rearrange("p (ci c) -> p ci c", ci=NCI),
                wq_d[:].rearrange("(ci p) c -> p ci c", p=128))
            nc.sync.dma_start(
                wk_all[:].rearrange("p (ci c) -> p ci c", ci=NCI),
                wk_d[:].rearrange("(ci p) c -> p ci c", p=128))
            nc.sync.dma_start(
                wv_all[:].rearrange("p (ci c) -> p ci c", ci=NCI),
                wv_d[:].rearrange("(ci p) c -> p ci c", p=128))
            nc.sync.dma_start(
                wp_all[:].rearrange("p (j c) -> p j c", j=4),
                wp_d[:].rearrange("(j p) c -> p j c", p=128))

            # AV matmuls and normalization are drained lazily from a queue
            # that survives across units, so one unit's AV tail interleaves
            # with the next unit's score/exp stream instead of leaving the
            # Scalar engine idle during a back-to-back AV burst.
            pend = []

            def emit_av_item(item):
                u, h01, kt2, c02, width2, es2 = item
                if _BENCH["av"]:
                    if u["avs"] is None:
                        av0 = psm.tile([65, 512], dt.float32, tag="av0")
                        av1 = psm.tile([128, 512], dt.float32, tag="av1")
                        u["avs"] = (av0, av1)
                    vofs = u["j"] * PAIR_BLK + h01 * 64
                    lw = 65 if h01 == 0 else 128
                    nc.tensor.matmul(
                        u["avs"][h01][:, c02:512],
                        vsb[kt2][:, vofs:vofs + lw],
                        es2[:, h01 * 512:h01 * 512 + width2],
                        start=(kt2 == 0), stop=(kt2 == u["n_kt"] - 1))
                if h01 == 1 and kt2 == u["n_kt"] - 1:
                    emit_norm(u)

            def emit_norm(u):
                if not (_BENCH["norm"] and _BENCH["av"]):
                    return
                av0, av1 = u["avs"]
                j, q0 = u["j"], u["q0"]
                sr = pm.tile([128, 512], dt.float32, tag="sr", bufs=1)
                nc.vector.tensor_copy(sr[64:65, :], av0[64:65, :])
                sr2 = pm.tile([1, 512], dt.float32, tag="sr2", bufs=2)
                nc.vector.tensor_copy(sr2[0:1, :], av1[0:1, :])
                ra = pm.tile([1, 512], dt.float32, tag="ra", bufs=2)
                nc.sync.dma_start(ra[0:1, :], sr[64:65, :])
                rra = pm.tile([1, 512], dt.float32, tag="rra", bufs=1)
                rrb = pm.tile([1, 512], dt.float32, tag="rrb", bufs=1)
                nc.vector.reciprocal_approx_fast(out=rra[0:1, :], in_=ra[0:1, :])
                nc.vector.reciprocal_approx_fast(out=rrb[0:1, :], in_=sr2[0:1, :])
                bca = pm.tile([128, 512], dt.float32, tag="bca", bufs=2)
                bcb = pm.tile([128, 512], dt.float32, tag="bcb", bufs=2)
                nc.gpsimd.partition_broadcast(bca[:, :], rra[0:1, :], channels=128)
                nc.gpsimd.partition_broadcast(bcb[:, :], rrb[0:1, :], channels=128)
                nc.vector.tensor_mul(ysb[j][0:64, q0:q0 + 512],
                                     av0[0:64, :], bca[0:64, :])
                nc.vector.tensor_mul(ysb[j][64:128, q0:q0 + 512],
                                     av1[64:128, :], bcb[64:128, :])

            def flush_pend():
                while pend:
                    emit_av_item(pend.pop(0))

            def emit_unit(qb, j, qtrj):
                q0 = qb * 512
                n_kt = qb * 4 + 4
                u = {"j": j, "q0": q0, "n_kt": n_kt, "avs": None}

                for kt in range(n_kt):
                    off = kt * 128 - q0
                    c0 = min(max(off, 0), 256)
                    width = 512 - c0
                    st = psm.tile([128, 1024], dt.float32, tag="st",
                                  bufs=ST_BUFS)
                    if _BENCH["scores"]:
                        for h01 in range(2):
                            hb = h01 * 64
                            nc.tensor.matmul(
                                st[:, h01 * 512:h01 * 512 + width],
                                kt_[j][hb:hb + 64, kt * 128:(kt + 1) * 128],
                                qtrj[hb:hb + 64, c0:512],
                                start=True, stop=True, tile_position=(hb, 0))
                    es = pm.tile([128, 1024], dta, tag="es",
                                 bufs=ES_BUFS)
                    if _BENCH["exp"] or _BENCH["scores"]:
                        # one fused exp over both heads' scores; the unread
                        # [width:512) gap holds exp(stale PSUM) and is never
                        # consumed (AV reads only [h01*512 : h01*512+width))
                        nc.scalar.activation(
                            es[:, 0:512 + width], st[:, 0:512 + width],
                            AF.Exp if _BENCH["exp"] else AF.Copy, scale=0.125)
                    if off >= 0 and _BENCH["mask"]:
                        # causal mask applied post-exp as a 0/1 multiply on
                        # the otherwise-idle Pool engine (SBUF bf16)
                        mw = off - c0 + 128
                        for h01 in range(2):
                            nc.gpsimd.tensor_tensor(
                                es[:, h01 * 512:h01 * 512 + mw],
                                es[:, h01 * 512:h01 * 512 + mw],
                                mask01[:, 256 - mw:256], Alu.mult)
                    for h01 in range(2):
                        pend.append((u, h01, kt, c0, width, es))
                    while len(pend) > 2 * AV_LAG:
                        emit_av_item(pend.pop(0))

            def emit_proj(tts):
                if not _BENCH["proj"]:
                    return
                for tt in tts:
                    po = pm.tile([128, C], dt.float32, tag="po", bufs=2)
                    for cb in range(2):
                        pj = psm.tile([128, 512], dt.float32, tag="vps", bufs=2)
                        for j in range(4):
                            wsl = wp_all[:, j * C + cb * 512:j * C + (cb + 1) * 512]
                            nc.tensor.matmul(
                                pj[:], ysb[j][:, tt * 128:(tt + 1) * 128], wsl,
                                start=(j == 0), stop=(j == 3))
                        nc.vector.tensor_copy(po[:, cb * 512:(cb + 1) * 512], pj[:])
                    nc.sync.dma_start(out_d[tt * 128:(tt + 1) * 128, :], po[:])

            prev_qtr = None
            xs_cur = xs0
            for tb in range(NB):
                ts = slice(tb * 512, (tb + 1) * 512)
                if tb > 0:
                    xs_cur = pm.tile([128, NCI * 512], dta, tag="xall", bufs=2)
                    nc.sync.dma_start(
                        xs_cur[:].rearrange("p (ci c) -> p ci c", ci=NCI),
                        xt_d[:, ts].rearrange("(ci p) c -> p ci c", p=128))
                # Q^T (rolling, this block only) and K^T (persistent)
                qtr = []
                for j in range(4):
                    ps = psm.tile([128, 512], dt.float32, tag="vps", bufs=2)
                    for ci in range(NCI):
                        nc.tensor.matmul(
                            ps[:],
                            wq_all[:, ci * CL + j * 128:ci * CL + (j + 1) * 128],
                            xs_cur[:, ci * 512:(ci + 1) * 512],
                            start=(ci == 0), stop=(ci == NCI - 1))
                    qj = pm.tile([128, 512], dta, tag=f"qtr{j}", name=f"qtr{j}", bufs=2)
                    nc.vector.tensor_scalar_add(qj[:], ps[:], bq_sb[:, j:j + 1])
                    qtr.append(qj)
                    if prev_qtr is not None:
                        emit_unit(tb - 1, j, prev_qtr[j])
                for j in range(4):
                    ps = psm.tile([128, 512], dt.float32, tag="vps", bufs=2)
                    for ci in range(NCI):
                        nc.tensor.matmul(
                            ps[:],
                            wk_all[:, ci * CL + j * 128:ci * CL + (j + 1) * 128],
                            xs_cur[:, ci * 512:(ci + 1) * 512],
                            start=(ci == 0), stop=(ci == NCI - 1))
                    nc.vector.tensor_scalar_add(kt_[j][:, ts], ps[:], bk_sb[:, j:j + 1])
                # V tiles for this block
                for tt in range(tb * 4, tb * 4 + 4):
                    lt = tt % 4
                    ps = psm.tile([128, CL], dt.float32, tag="vps", bufs=2)
                    for ci in range(NCI):
                        nc.tensor.matmul(
                            ps[:],
                            xs_cur[:, ci * 512 + lt * 128:ci * 512 + (lt + 1) * 128],
                            wv_all[:, ci * CL:(ci + 1) * CL],
                            start=(ci == 0), stop=False)
                    nc.tensor.matmul(ps[:], ones[0:1, :], bvr[:],
                                     start=False, stop=True)
                    vt = vsb[tt]
                    vmc = vm16[:, tt:tt + 1]
                    ve_out = vt[:].rearrange("p (q b) -> p q b", b=PAIR_BLK)[:, :, 0:64]
                    ve_in = ps[:].rearrange("p (q b) -> p q b", b=128)[:, :, 0:64]
                    nc.vector.tensor_scalar_mul(ve_out, ve_in, vmc)
                    vo_out = vt[:].rearrange("p (q b) -> p q b", b=PAIR_BLK)[:, :, 128:192]
                    vo_in = ps[:].rearrange("p (q b) -> p q b", b=128)[:, :, 64:128]
                    nc.vector.tensor_scalar_mul(vo_out, vo_in, vmc)
                    for p_ in range(4):
                        nc.vector.tensor_copy(vt[:, p_ * PAIR_BLK + 64:p_ * PAIR_BLK + 65],
                                              vmc)
                if tb == NB - 1 and NB > 1:
                    emit_proj(range(0, 3))
                prev_qtr = qtr

            # ---- tail: last-block attention interleaved with the projection ----
            # proj for blocks qb <= NB-2 interleaves with the tail units;
            # the last block's tiles go after its final unit
            done = (NB - 1) * 4  # y rows complete pre-tail (0..3 emitted in-loop)
            base = 3 if NB > 1 else 0
            for j in range(4):
                emit_unit(NB - 1, j, prev_qtr[j])
                if j < 3 and done > base:
                    lo = base + j * (done - base) // 3
                    hi = base + (j + 1) * (done - base) // 3
                    emit_proj(range(lo, hi))
            flush_pend()
            emit_proj(range(max(done, base) if NB > 1 else 0, NT))


def _shard_inputs(x, attention_mask, Wq, bq, Wk, bk, Wv, bv, Wp, t_len):
    big = np.float32(-3.0e38)
    mka = np.full((128, 256), big, np.float32)
    r_, c_ = np.arange(128)[:, None], np.arange(128)[None, :]
    mka[:, 128:256] = np.where(c_ >= r_, np.float32(0.0), big)
    ones = _f32r_round(np.ones((128, 128), np.float32))
    in_maps = []
    for core in range(8):
        b, hg = core // 2, core % 2
        hs = slice(hg * CL, (hg + 1) * CL)
        in_maps.append({
            "xt": _f32r_round(x[b, :t_len].T),
            "wq": _f32r_round(Wq[:, hs]),
            "wk": _f32r_round(Wk[:, hs]),
            "wv": _f32r_round(Wv[:, hs]),
            "wp": _f32r_round(Wp[hs, :]),
            "bq": np.ascontiguousarray(bq[hs], np.float32).reshape(CL, 1),
            "bk": np.ascontiguousarray(bk[hs], np.float32).reshape(CL, 1),
            "bvr": _f32r_round(bv[hs].reshape(1, CL)),
            "vm": np.ascontiguousarray(
                attention_mask[b, :t_len].astype(np.float32).reshape(t_len // 128, 128).T),
            "mka": mka,
            "mkb": mkb,
            "ones": ones,
        })
    return in_maps


def kernel(**inputs):
    from concourse import bass_utils

    t_len = T
    key = ("nc", t_len)
    if key not in _CACHE:
        _CACHE[key] = _build(t_len)
    nc = _CACHE[key]

    x = np.asarray(inputs["x"], dtype=np.float32)
    am = np.asarray(inputs["attention_mask"])
    in_maps = _shard_inputs(
        x, am, np.asarray(inputs["Wq"], np.float32), np.asarray(inputs["bq"], np.float32),
        np.asarray(inputs["Wk"], np.float32), np.asarray(inputs["bk"], np.float32),
        np.asarray(inputs["Wv"], np.float32), np.asarray(inputs["bv"], np.float32),
        np.asarray(inputs["Wp"], np.float32), t_len)

    res = bass_utils.run_bass_kernel_spmd(nc, in_maps, core_ids=list(range(8)))
    bp = np.asarray(inputs["bp"], np.float32)
    out = np.empty((B, T, C), dtype=np.float32)
    for b in range(B):
        out[b] = res.results[2 * b]["out"] + res.results[2 * b + 1]["out"] + bp
    return out



# revision 25
# speedup vs baseline: 1.3311x; 1.3311x over previous
"""Trainium2 Bass kernel for causal multi-head attention (B=4, T=2048, C=1024, H=16).

Sharding: 8 NeuronCores = batch (4) x head-group (2). Each core computes, for
its batch b and its 8 heads:
  - QKV projections with column-sharded weights (Q^T/K^T in [D*,T] layout,
    V in [T, D*] layout),
  - causal attention with an appended validity/row-sum column on V
    (flash-style unnormalized accumulation + fused denominator),
  - row-sharded output projection producing a partial [T, C] output.
The host sums the two head-group partials per batch and adds the output bias.

All matmul operands are bf16 (fp32 PSUM accumulation; rel err ~5e-3, well
inside the 2e-2 gate); operands are cast host-side. Weight/x-block loads are
single batched DMAs (3D access patterns), y^T stays resident in SBUF (no
DRAM bounce), and the per-row-tile output is written with one DMA — DMA
issue on the sync queue was the serial bottleneck at 204 DMAs/exec.

Schedule: one rolling loop — attention for query-block qb is emitted
interleaved with the projections of block qb+1, so the exp-bound Scalar-engine
stretches hide under PE-bound projection matmuls (Q^T rolls per block; the
output projection reads SBUF-resident y^T and interleaves with the last
block's attention). Attention-V matmuls lag the score/exp stream (AV_LAG) to
keep the in-order PE queue from head-of-line blocking on the Scalar engine.

`_build(t_len, repeat=K)` emits the whole body K times into one NEFF; test.py
uses that for loop-timing on hardware (the axon tunnel's ~2-15 ms per-call
dispatch floor would otherwise swamp the ~0.44 ms device time).
"""

import numpy as np
from contextlib import ExitStack

B, T, C, H = 4, 2048, 1024, 16
D = C // H            # 64
CL = C // 2           # 512 local channels (8 heads) per core
NCI = C // 128        # 8 contraction tiles for projections
PAIR_BLK = 192        # v_sb columns per head pair: [V_e(64) | valid(1) | gap(63) | V_o(64)]

_CACHE = {}

# schedule-pipelining knobs
AV_LAG = 10
ST_BUFS = 2
ES_BUFS = 12

# timing-variant switches (bench-only; kernel() always runs the full body)
_BENCH = dict(scores=True, mask=True, exp=True, av=True, norm=True,
              spill=True, proj=True)

# matmul-operand precision: "bf16" (half SBUF/DMA traffic) or "f32r"
_PREC = "bf16"


def _f32r_round(a):
    """Round fp32 -> float32r bit pattern (keep top 12 mantissa bits, round half up)."""
    a = np.ascontiguousarray(a, dtype=np.float32)
    u = a.view(np.uint32).astype(np.uint64)
    r = ((u + 0x7FF + ((u >> 12) & 1)) & 0xFFFFF000).astype(np.uint32)
    return r.view(np.float32).reshape(a.shape)


def _cast_op(a):
    """Host-side cast of a matmul operand to the active precision."""
    if _PREC == "bf16":
        import ml_dtypes
        return np.ascontiguousarray(a, dtype=np.float32).astype(ml_dtypes.bfloat16)
    return _f32r_round(a)


def _build(t_len, repeat=1):
    import concourse.bass as bass  # noqa: F401
    import concourse.tile as tile
    from concourse import bacc, mybir

    dt = mybir.dt
    AF = mybir.ActivationFunctionType
    Alu = mybir.AluOpType

    NT = t_len // 128     # t tiles
    NB = t_len // 512     # t blocks

    dta = dt.bfloat16 if _PREC == "bf16" else dt.float32r

    nc = bacc.Bacc("TRN2", target_bir_lowering=False, debug=False,
                   enable_asserts=False, num_devices=8)

    xt_d = nc.dram_tensor("xt", (C, t_len), dta, kind="ExternalInput").ap()
    wq_d = nc.dram_tensor("wq", (C, CL), dta, kind="ExternalInput").ap()
    wk_d = nc.dram_tensor("wk", (C, CL), dta, kind="ExternalInput").ap()
    wv_d = nc.dram_tensor("wv", (C, CL), dta, kind="ExternalInput").ap()
    wp_d = nc.dram_tensor("wp", (CL, C), dta, kind="ExternalInput").ap()
    bq_d = nc.dram_tensor("bq", (CL, 1), dt.float32, kind="ExternalInput").ap()
    bk_d = nc.dram_tensor("bk", (CL, 1), dt.float32, kind="ExternalInput").ap()
    bvr_d = nc.dram_tensor("bvr", (1, CL), dta, kind="ExternalInput").ap()
    vm_d = nc.dram_tensor("vm", (128, NT), dt.float32, kind="ExternalInput").ap()
    mka_d = nc.dram_tensor("mka", (128, 256), dt.float32, kind="ExternalInput").ap()
    ones_d = nc.dram_tensor("ones", (128, 128), dta, kind="ExternalInput").ap()
    out_d = nc.dram_tensor("out", (t_len, C), dt.float32, kind="ExternalOutput").ap()

    with tile.TileContext(nc) as tc:
        for _rep in range(repeat):
            _emit_body(nc, tile, dt, AF, Alu, NT, NB, t_len, _rep,
                       xt_d, wq_d, wk_d, wv_d, wp_d, bq_d, bk_d, bvr_d,
                       vm_d, mka_d, ones_d, out_d, tc)

    nc.compile()
    return nc


def _emit_body(nc, tile, dt, dta, AF, Alu, NT, NB, t_len, rep,
               xt_d, wq_d, wk_d, wv_d, wp_d, bq_d, bk_d, bvr_d,
               vm_d, mka_d, ones_d, out_d, tc):
    with ExitStack() as octx:
        persist = octx.enter_context(tc.tile_pool(name=f"persist{rep}", bufs=1))

        # Small persistent tensors
        maskadd = persist.tile([128, 256], dt.float32, tag="mka")
        nc.sync.dma_start(maskadd[:], mka_d[:])
        ones = persist.tile([128, 128], dta, tag="ones")
        nc.sync.dma_start(ones[:], ones_d[:])
        vm16 = persist.tile([128, NT], dt.float32, tag="vm16")
        nc.sync.dma_start(vm16[:], vm_d[:])
        bvr = persist.tile([1, CL], dta, tag="bvr")
        nc.sync.dma_start(bvr[:], bvr_d[:])
        bq_sb = persist.tile([128, 4], dt.float32, tag="bq")
        bk_sb = persist.tile([128, 4], dt.float32, tag="bk")
        nc.sync.dma_start(bq_sb[:].rearrange("p (j one) -> p j one", one=1),
                          bq_d[:].rearrange("(j p) one -> p j one", p=128))
        nc.sync.dma_start(bk_sb[:].rearrange("p (j one) -> p j one", one=1),
                          bk_d[:].rearrange("(j p) one -> p j one", p=128))

        # Persistent activations (Q^T is rolled per t-block; K^T/V/y^T persist)
        kt_ = [persist.tile([128, t_len], dta, tag=f"kt{j}", name=f"kt{j}") for j in range(4)]
        vsb = [persist.tile([128, 4 * PAIR_BLK], dta, tag=f"v{t}",
                             name=f"v{t}") for t in range(NT)]
        ysb = [persist.tile([128, t_len], dta, tag=f"y{j}", name=f"ysb{j}")
               for j in range(4)]

        # ------- merged loop: projections for t-block tb, then attention qb=tb -------
        with (
            tc.tile_pool(name="pm", bufs=1) as pm,
            tc.tile_pool(name="psm", bufs=1, space="PSUM") as psm,
        ):
            # one batched DMA per weight matrix / x block
            wq_all = pm.tile([128, NCI * CL], dta, tag="wqa", name="wq_all")
            wk_all = pm.tile([128, NCI * CL], dta, tag="wka", name="wk_all")
            wv_all = pm.tile([128, NCI * CL], dta, tag="wva", name="wv_all")
            wp_all = pm.tile([128, 4 * C], dta, tag="wpa", name="wp_all")
            xs0 = pm.tile([128, NCI * 512], dta, tag="xall", name="xs0", bufs=2)
            nc.sync.dma_start(
                xs0[:].rearrange("p (ci c) -> p ci c", ci=NCI),
                xt_d[:, 0:512].rearrange("(ci p) c -> p ci c", p=128))
            nc.sync.dma_start(
                wq_all[:].rearrange("p (ci c) -> p ci c", ci=NCI),
                wq_d[:].rearrange("(ci p) c -> p ci c", p=128))
            nc.sync.dma_start(
                wk_all[:].rearrange("p (ci c) -> p ci c", ci=NCI),
                wk_d[:].rearrange("(ci p) c -> p ci c", p=128))
            nc.sync.dma_start(
                wv_all[:].rearrange("p (ci c) -> p ci c", ci=NCI),
                wv_d[:].rearrange("(ci p) c -> p ci c", p=128))
            nc.sync.dma_start(
                wp_all[:].rearrange("p (j c) -> p j c", j=4),
                wp_d[:].rearrange("(j p) c -> p j c", p=128))

            # AV matmuls and normalization are drained lazily from a queue
            # that survives across units, so one unit's AV tail interleaves
            # with the next unit's score/exp stream instead of leaving the
            # Scalar engine idle during a back-to-back AV burst.
            pend = []

            def emit_av_item(item):
                u, h01, kt2, c02, width2, es2 = item
                if _BENCH["av"]:
                    if u["avs"] is None:
                        av0 = psm.tile([65, 512], dt.float32, tag="av0")
                        av1 = psm.tile([128, 512], dt.float32, tag="av1")
                        u["avs"] = (av0, av1)
                    vofs = u["j"] * PAIR_BLK + h01 * 64
                    lw = 65 if h01 == 0 else 128
                    nc.tensor.matmul(
                        u["avs"][h01][:, c02:512],
                        vsb[kt2][:, vofs:vofs + lw],
                        es2[:, h01 * 512:h01 * 512 + width2],
                        start=(kt2 == 0), stop=(kt2 == u["n_kt"] - 1))
                if h01 == 1 and kt2 == u["n_kt"] - 1:
                    emit_norm(u)

            def emit_norm(u):
                if not (_BENCH["norm"] and _BENCH["av"]):
                    return
                av0, av1 = u["avs"]
                j, q0 = u["j"], u["q0"]
                sr = pm.tile([128, 512], dt.float32, tag="sr", bufs=1)
                nc.vector.tensor_copy(sr[64:65, :], av0[64:65, :])
                sr2 = pm.tile([1, 512], dt.float32, tag="sr2", bufs=2)
                nc.vector.tensor_copy(sr2[0:1, :], av1[0:1, :])
                ra = pm.tile([1, 512], dt.float32, tag="ra", bufs=2)
                nc.sync.dma_start(ra[0:1, :], sr[64:65, :])
                rra = pm.tile([1, 512], dt.float32, tag="rra", bufs=1)
                rrb = pm.tile([1, 512], dt.float32, tag="rrb", bufs=1)
                nc.vector.reciprocal_approx_fast(out=rra[0:1, :], in_=ra[0:1, :])
                nc.vector.reciprocal_approx_fast(out=rrb[0:1, :], in_=sr2[0:1, :])
                bca = pm.tile([128, 512], dt.float32, tag="bca", bufs=2)
                bcb = pm.tile([128, 512], dt.float32, tag="bcb", bufs=2)
                nc.gpsimd.partition_broadcast(bca[:, :], rra[0:1, :], channels=128)
                nc.gpsimd.partition_broadcast(bcb[:, :], rrb[0:1, :], channels=128)
                nc.vector.tensor_mul(ysb[j][0:64, q0:q0 + 512],
                                     av0[0:64, :], bca[0:64, :])
                nc.vector.tensor_mul(ysb[j][64:128, q0:q0 + 512],
                                     av1[64:128, :], bcb[64:128, :])

            def flush_pend():
                while pend:
                    emit_av_item(pend.pop(0))

            def emit_unit(qb, j, qtrj):
                q0 = qb * 512
                n_kt = qb * 4 + 4
                u = {"j": j, "q0": q0, "n_kt": n_kt, "avs": None}

                for kt in range(n_kt):
                    off = kt * 128 - q0
                    c0 = min(max(off, 0), 256)
                    width = 512 - c0
                    st = psm.tile([128, 1024], dt.float32, tag="st",
                                  bufs=ST_BUFS)
                    if _BENCH["scores"]:
                        for h01 in range(2):
                            hb = h01 * 64
                            nc.tensor.matmul(
                                st[:, h01 * 512:h01 * 512 + width],
                                kt_[j][hb:hb + 64, kt * 128:(kt + 1) * 128],
                                qtrj[hb:hb + 64, c0:512],
                                start=True, stop=True, tile_position=(hb, 0))
                    if off >= 0 and _BENCH["mask"]:
                        mw = off - c0 + 128
                        for h01 in range(2):
                            nc.vector.tensor_tensor(
                                st[:, h01 * 512:h01 * 512 + mw],
                                st[:, h01 * 512:h01 * 512 + mw],
                                maskadd[:, 256 - mw:256], Alu.add)
                    es = pm.tile([128, 1024], dta, tag="es",
                                 bufs=ES_BUFS)
                    if _BENCH["exp"] or _BENCH["scores"]:
                        # one fused exp over both heads' scores; the unread
                        # [width:512) gap holds exp(stale PSUM) and is never
                        # consumed (AV reads only [h01*512 : h01*512+width))
                        nc.scalar.activation(
                            es[:, 0:512 + width], st[:, 0:512 + width],
                            AF.Exp if _BENCH["exp"] else AF.Copy, scale=0.125)
                    for h01 in range(2):
                        pend.append((u, h01, kt, c0, width, es))
                    while len(pend) > 2 * AV_LAG:
                        emit_av_item(pend.pop(0))

            def emit_proj(tts):
                if not _BENCH["proj"]:
                    return
                for tt in tts:
                    po = pm.tile([128, C], dt.float32, tag="po", bufs=2)
                    for cb in range(2):
                        pj = psm.tile([128, 512], dt.float32, tag="vps", bufs=2)
                        for j in range(4):
                            wsl = wp_all[:, j * C + cb * 512:j * C + (cb + 1) * 512]
                            nc.tensor.matmul(
                                pj[:], ysb[j][:, tt * 128:(tt + 1) * 128], wsl,
                                start=(j == 0), stop=(j == 3))
                        nc.vector.tensor_copy(po[:, cb * 512:(cb + 1) * 512], pj[:])
                    nc.sync.dma_start(out_d[tt * 128:(tt + 1) * 128, :], po[:])

            prev_qtr = None
            xs_cur = xs0
            for tb in range(NB):
                ts = slice(tb * 512, (tb + 1) * 512)
                if tb > 0:
                    xs_cur = pm.tile([128, NCI * 512], dta, tag="xall", bufs=2)
                    nc.sync.dma_start(
                        xs_cur[:].rearrange("p (ci c) -> p ci c", ci=NCI),
                        xt_d[:, ts].rearrange("(ci p) c -> p ci c", p=128))
                # Q^T (rolling, this block only) and K^T (persistent)
                qtr = []
                for j in range(4):
                    ps = psm.tile([128, 512], dt.float32, tag="vps", bufs=2)
                    for ci in range(NCI):
                        nc.tensor.matmul(
                            ps[:],
                            wq_all[:, ci * CL + j * 128:ci * CL + (j + 1) * 128],
                            xs_cur[:, ci * 512:(ci + 1) * 512],
                            start=(ci == 0), stop=(ci == NCI - 1))
                    qj = pm.tile([128, 512], dta, tag=f"qtr{j}", name=f"qtr{j}", bufs=2)
                    nc.vector.tensor_scalar_add(qj[:], ps[:], bq_sb[:, j:j + 1])
                    qtr.append(qj)
                    if prev_qtr is not None:
                        emit_unit(tb - 1, j, prev_qtr[j])
                for j in range(4):
                    ps = psm.tile([128, 512], dt.float32, tag="vps", bufs=2)
                    for ci in range(NCI):
                        nc.tensor.matmul(
                            ps[:],
                            wk_all[:, ci * CL + j * 128:ci * CL + (j + 1) * 128],
                            xs_cur[:, ci * 512:(ci + 1) * 512],
                            start=(ci == 0), stop=(ci == NCI - 1))
                    nc.vector.tensor_scalar_add(kt_[j][:, ts], ps[:], bk_sb[:, j:j + 1])
                # V tiles for this block
                for tt in range(tb * 4, tb * 4 + 4):
                    lt = tt % 4
                    ps = psm.tile([128, CL], dt.float32, tag="vps", bufs=2)
                    for ci in range(NCI):
                        nc.tensor.matmul(
                            ps[:],
                            xs_cur[:, ci * 512 + lt * 128:ci * 512 + (lt + 1) * 128],
                            wv_all[:, ci * CL:(ci + 1) * CL],
                            start=(ci == 0), stop=False)
                    nc.tensor.matmul(ps[:], ones[0:1, :], bvr[:],
                                     start=False, stop=True)
                    vt = vsb[tt]
                    vmc = vm16[:, tt:tt + 1]
                    ve_out = vt[:].rearrange("p (q b) -> p q b", b=PAIR_BLK)[:, :, 0:64]
                    ve_in = ps[:].rearrange("p (q b) -> p q b", b=128)[:, :, 0:64]
                    nc.vector.tensor_scalar_mul(ve_out, ve_in, vmc)
                    vo_out = vt[:].rearrange("p (q b) -> p q b", b=PAIR_BLK)[:, :, 128:192]
                    vo_in = ps[:].rearrange("p (q b) -> p q b", b=128)[:, :, 64:128]
                    nc.vector.tensor_scalar_mul(vo_out, vo_in, vmc)
                    for p_ in range(4):
                        nc.vector.tensor_copy(vt[:, p_ * PAIR_BLK + 64:p_ * PAIR_BLK + 65],
                                              vmc)
                if tb == NB - 1 and NB > 1:
                    emit_proj(range(0, 3))
                prev_qtr = qtr

            # ---- tail: last-block attention interleaved with the projection ----
            # proj for blocks qb <= NB-2 interleaves with the tail units;
            # the last block's tiles go after its final unit
            done = (NB - 1) * 4  # y rows complete pre-tail (0..3 emitted in-loop)
            base = 3 if NB > 1 else 0
            for j in range(4):
                emit_unit(NB - 1, j, prev_qtr[j])
                if j < 3 and done > base:
                    lo = base + j * (done - base) // 3
                    hi = base + (j + 1) * (done - base) // 3
                    emit_proj(range(lo, hi))
            flush_pend()
            emit_proj(range(max(done, base) if NB > 1 else 0, NT))


def _shard_inputs(x, attention_mask, Wq, bq, Wk, bk, Wv, bv, Wp, t_len):
    big = np.float32(-3.0e38)
    mka = np.full((128, 256), big, np.float32)
    r_, c_ = np.arange(128)[:, None], np.arange(128)[None, :]
    mka[:, 128:256] = np.where(c_ >= r_, np.float32(0.0), big)
    ones = _f32r_round(np.ones((128, 128), np.float32))
    in_maps = []
    for core in range(8):
        b, hg = core // 2, core % 2
        hs = slice(hg * CL, (hg + 1) * CL)
        in_maps.append({
            "xt": _f32r_round(x[b, :t_len].T),
            "wq": _f32r_round(Wq[:, hs]),
            "wk": _f32r_round(Wk[:, hs]),
            "wv": _f32r_round(Wv[:, hs]),
            "wp": _f32r_round(Wp[hs, :]),
            "bq": np.ascontiguousarray(bq[hs], np.float32).reshape(CL, 1),
            "bk": np.ascontiguousarray(bk[hs], np.float32).reshape(CL, 1),
            "bvr": _f32r_round(bv[hs].reshape(1, CL)),
            "vm": np.ascontiguousarray(
                attention_mask[b, :t_len].astype(np.float32).reshape(t_len // 128, 128).T),
            "mka": mka,
            "ones": ones,
        })
    return in_maps


def kernel(**inputs):
    from concourse import bass_utils

    t_len = T
    key = ("nc", t_len)
    if key not in _CACHE:
        _CACHE[key] = _build(t_len)
    nc = _CACHE[key]

    x = np.asarray(inputs["x"], dtype=np.float32)
    am = np.asarray(inputs["attention_mask"])
    in_maps = _shard_inputs(
        x, am, np.asarray(inputs["Wq"], np.float32), np.asarray(inputs["bq"], np.float32),
        np.asarray(inputs["Wk"], np.float32), np.asarray(inputs["bk"], np.float32),
        np.asarray(inputs["Wv"], np.float32), np.asarray(inputs["bv"], np.float32),
        np.asarray(inputs["Wp"], np.float32), t_len)

    res = bass_utils.run_bass_kernel_spmd(nc, in_maps, core_ids=list(range(8)))
    bp = np.asarray(inputs["bp"], np.float32)
    out = np.empty((B, T, C), dtype=np.float32)
    for b in range(B):
        out[b] = res.results[2 * b]["out"] + res.results[2 * b + 1]["out"] + bp
    return out

